# revision 2
# baseline (speedup 1.0000x reference)
"""Trainium2 Bass kernel for the nn_CA depthwise-conv CA step (v3, bf16).

Pipeline per image: depthwise 5x5 conv (D4-symmetrized, zero-mean kernel,
SAME padding) + leaky_relu (bias1==0 in this problem); 1x1 conv (8x8
channel GEMM) + bias + leaky_relu; 1x1 conv + bias + psi residual + tanh.

Strategy: pure data parallel over 8 NeuronCores (256 images each), bf16
data (tolerance 2e-2 rel).

Two on-chip layouts:
  A: partition p=(b, y) for channel pair q (2q+b), free=(image, x). Conv
     stage runs here: 3 matmuls per pair (banded-Toeplitz lhsT for the
     y-conv; D4 x-symmetry folds the 5 x-taps into center + s13 + s2,
     where s13 = psi(x-1)+psi(x+1), s2 = psi(x-2)+psi(x+2) are computed
     on the DVE from the x-padded psi tile — no extra DMA).
  B: partition p=(c, y16), free=(ybl, image, x) per yb-pair tile. Stage2's
     lhsT permutes A->B for free; stage3 then contracts all 8 channels in
     ONE matmul per 512-col tile (4 instead of 16). Residual psi is DMA'd
     a second time in layout B (bf16, cheap) and added on the DVE.

ACT work is merged into 1024-col activations (6 per S8-step).
"""

import numpy as np
import ml_dtypes

BS, H, RES = 2048, 8, 64
NCORES = 8
IPC = BS // NCORES  # images per core
SG = 16             # images per super-group (DMA granularity)
NSG = IPC // SG
S8 = 8              # images per PSUM step (512 free columns)
NST = SG // S8
XP = RES + 4        # x-padded width
NW = 29             # lhsT matrices

_CACHE = {}


def _totalistic(x):
    # D4-symmetrize 5x5 kernels over spatial dims, then remove spatial mean
    z = 0.125 * (x + x[:, :, ::-1, :] + x[:, :, :, ::-1] + x[:, :, ::-1, ::-1])
    xt = np.swapaxes(x, 2, 3)
    z = z + 0.125 * (xt + xt[:, :, ::-1, :] + xt[:, :, :, ::-1] + xt[:, :, ::-1, ::-1])
    return z - z.mean(axis=(2, 3), keepdims=True)


def _build_program(reps=1):
    import concourse.bacc as bacc
    import concourse.tile as tile
    from concourse import mybir

    dt = mybir.dt
    nc = bacc.Bacc("TRN2", target_bir_lowering=False, debug=False, num_devices=NCORES)
    pa = nc.dram_tensor("pa", [4, 128, IPC * XP], dt.bfloat16, kind="ExternalInput").ap()
    pb = nc.dram_tensor("pb", [2, 128, IPC * 2 * RES], dt.bfloat16, kind="ExternalInput").ap()
    wm = nc.dram_tensor("wm", [NW, 128, 128], dt.bfloat16, kind="ExternalInput").ap()
    bv = nc.dram_tensor("bv", [128, 2], dt.float32, kind="ExternalInput").ap()
    out = nc.dram_tensor("out", [2, 128, IPC * 2 * RES], dt.bfloat16, kind="ExternalOutput").ap()

    LR = mybir.ActivationFunctionType.Lrelu
    TH = mybir.ActivationFunctionType.Tanh

    with tile.TileContext(nc) as tc:
        from contextlib import ExitStack

        with ExitStack() as ctx:
            const = ctx.enter_context(tc.tile_pool(name="const", bufs=1))
            psip = ctx.enter_context(tc.tile_pool(name="psip", bufs=2))
            spool = ctx.enter_context(tc.tile_pool(name="spool", bufs=2))
            opool = ctx.enter_context(tc.tile_pool(name="opool", bufs=2))
            zpool = ctx.enter_context(tc.tile_pool(name="zpool", bufs=2))
            psum = ctx.enter_context(tc.tile_pool(name="psum", bufs=4, space="PSUM"))

            wt = const.tile([128, NW * 128], dt.bfloat16)
            nc.sync.dma_start(
                wt[:].rearrange("p (m k) -> p m k", m=NW),
                wm.rearrange("m p k -> p m k"),
            )
            bt = const.tile([128, 2], dt.float32)
            nc.sync.dma_start(bt[:], bv[:])

            def W(i):
                return wt[:, i * 128 : (i + 1) * 128]

            for rep in range(reps):
              for sg in range(NSG):
                ptiles = []
                for q in range(4):
                    t = psip.tile([128, SG * XP], dt.bfloat16, tag=f"psi{q}",
                                  name=f"pa_t{q}_{sg}_{rep}")
                    nc.sync.dma_start(
                        t[:], pa[q, :, sg * SG * XP : (sg + 1) * SG * XP])
                    ptiles.append(t)
                pbtiles = []
                for yp in range(2):
                    t = psip.tile([128, SG * 2 * RES], dt.bfloat16, tag=f"pb{yp}",
                                  name=f"pb_t{yp}_{sg}_{rep}")
                    nc.sync.dma_start(
                        t[:], pb[yp, :, sg * SG * 2 * RES : (sg + 1) * SG * 2 * RES])
                    pbtiles.append(t)
                # x-symmetric partial sums on DVE (replaces the s2 DMA input)
                stiles = []
                for q in range(4):
                    pa3 = ptiles[q][:].rearrange("p (i x) -> p i x", i=SG)
                    s13 = spool.tile([128, SG * RES], dt.bfloat16, tag=f"s13_{q}",
                                     name=f"s13_{q}_{sg}_{rep}")
                    nc.vector.tensor_add(
                        s13[:].rearrange("p (i x) -> p i x", i=SG),
                        pa3[:, :, 1 : 1 + RES], pa3[:, :, 3 : 3 + RES])
                    s2 = spool.tile([128, SG * RES], dt.bfloat16, tag=f"s2_{q}",
                                    name=f"s2_{q}_{sg}_{rep}")
                    nc.vector.tensor_add(
                        s2[:].rearrange("p (i x) -> p i x", i=SG),
                        pa3[:, :, 0 : 0 + RES], pa3[:, :, 4 : 4 + RES])
                    stiles.append((s13, s2))
                otiles = [
                    opool.tile([128, SG * 2 * RES], dt.bfloat16, tag=f"o{yp}",
                               name=f"o_t{yp}_{sg}_{rep}")
                    for yp in range(2)
                ]
                for st in range(NST):
                    # stage 1: depthwise conv + lrelu (layout A)
                    z1half = []
                    for jj in range(2):
                        ps1 = psum.tile([128, 2 * S8 * RES], dt.float32, tag="ps",
                                        name=f"p1_{sg}_{st}_{jj}_{rep}")
                        for h in range(2):
                            q = 2 * jj + h
                            seg = ps1[:, h * S8 * RES : (h + 1) * S8 * RES].rearrange(
                                "p (i x) -> p i x", i=S8)
                            pt3 = ptiles[q][:].rearrange("p (i x) -> p i x", i=SG)[
                                :, st * S8 : (st + 1) * S8, :]
                            s13v = stiles[q][0][:].rearrange("p (i x) -> p i x", i=SG)[
                                :, st * S8 : (st + 1) * S8, :]
                            s2v = stiles[q][1][:].rearrange("p (i x) -> p i x", i=SG)[
                                :, st * S8 : (st + 1) * S8, :]
                            nc.tensor.matmul(seg, W(q * 3 + 2),
                                             pt3[:, :, 2 : 2 + RES],
                                             start=True, stop=False)
                            nc.tensor.matmul(seg, W(q * 3 + 1), s13v,
                                             start=False, stop=False)
                            nc.tensor.matmul(seg, W(q * 3 + 0), s2v,
                                             start=False, stop=True)
                        z = zpool.tile([128, 2 * S8 * RES], dt.bfloat16,
                                       tag=f"z1_{jj}", name=f"z1_{sg}_{st}_{jj}_{rep}")
                        nc.scalar.activation(z[:], ps1[:], LR, alpha=0.01)
                        z1half.append(z)
                    z1 = [
                        z1half[q // 2][:, (q % 2) * S8 * RES : (q % 2 + 1) * S8 * RES]
                        for q in range(4)
                    ]
                    # stage 2: 8x8 GEMM, layout A -> B via lhsT, + b2 + lrelu
                    z2t = []
                    for yp in range(2):
                        ps2 = psum.tile([128, 2 * S8 * RES], dt.float32, tag="ps",
                                        name=f"p2_{sg}_{st}_{yp}_{rep}")
                        for ybl in range(2):
                            yb = 2 * yp + ybl
                            seg = ps2[:, ybl * S8 * RES : (ybl + 1) * S8 * RES]
                            for q in range(4):
                                nc.tensor.matmul(seg, W(12 + yb * 4 + q), z1[q],
                                                 start=(q == 0), stop=(q == 3))
                        z = zpool.tile([128, 2 * S8 * RES], dt.bfloat16,
                                       tag=f"z2_{yp}", name=f"z2_{sg}_{st}_{yp}_{rep}")
                        nc.scalar.activation(z[:], ps2[:], LR,
                                             bias=bt[:, 0:1], alpha=0.01)
                        z2t.append(z)
                    # stage 3: 8x8 GEMM in layout B (1 matmul per 512-col tile)
                    # + psi residual (DVE) + b3 + tanh
                    for yp in range(2):
                        ps3 = psum.tile([128, 2 * S8 * RES], dt.float32, tag="ps",
                                        name=f"p3_{sg}_{st}_{yp}_{rep}")
                        for ybl in range(2):
                            seg = ps3[:, ybl * S8 * RES : (ybl + 1) * S8 * RES]
                            nc.tensor.matmul(
                                seg, W(28),
                                z2t[yp][:, ybl * S8 * RES : (ybl + 1) * S8 * RES],
                                start=True, stop=True)
                        pbv = pbtiles[yp][:, st * 2 * S8 * RES : (st + 1) * 2 * S8 * RES]
                        nc.vector.tensor_add(ps3[:], ps3[:], pbv)
                        nc.scalar.activation(
                            otiles[yp][:, st * 2 * S8 * RES : (st + 1) * 2 * S8 * RES],
                            ps3[:], TH, bias=bt[:, 1:2])
                for yp in range(2):
                    nc.sync.dma_start(
                        out[yp, :, sg * SG * 2 * RES : (sg + 1) * SG * 2 * RES],
                        otiles[yp][:])

    nc.compile()
    return nc


def _host_pack(filter1, bias1, w2, b2, w3, b3):
    w = _totalistic(filter1.astype(np.float32))[:, 0]  # [8,5,5]
    wm = np.zeros((NW, 128, 128), np.float32)
    eye = {d: np.eye(RES, k=-d, dtype=np.float32) for d in range(-2, 3)}
    for q in range(4):
        for dxi in range(3):
            m = wm[q * 3 + dxi]
            for b in range(2):
                blk = np.zeros((RES, RES), np.float32)
                for d in range(-2, 3):
                    blk += w[2 * q + b, d + 2, dxi] * eye[d]
                m[b * 64 : b * 64 + 64, b * 64 : b * 64 + 64] = blk
    # stage2: lhsT[(b, y64), (co, y16)] = w2[co, 2q+b] iff y64 == 16*yb + y16
    for yb in range(4):
        for q in range(4):
            m = wm[12 + yb * 4 + q]
            for b in range(2):
                for co in range(H):
                    for y16 in range(16):
                        m[b * 64 + 16 * yb + y16, co * 16 + y16] = w2[co, 2 * q + b]
    # stage3: lhsT[(ci, y16), (co, y16)] = w3[co, ci]
    m = wm[28]
    for ci in range(H):
        for co in range(H):
            for y16 in range(16):
                m[ci * 16 + y16, co * 16 + y16] = w3[co, ci]

    bvv = np.zeros((128, 2), np.float32)
    for c in range(H):
        bvv[c * 16 : c * 16 + 16, 0] = b2[c]
        bvv[c * 16 : c * 16 + 16, 1] = b3[c]
    return wm.astype(ml_dtypes.bfloat16), bvv


def _pack_psi_a(psi):
    """[BS,H,RES,RES] -> [NCORES, 4, 128, IPC*XP] bf16: partition p=(b,y) of
    channel pair q, free = (image, padded x)."""
    psip = np.zeros((BS, H, RES, XP), ml_dtypes.bfloat16)
    psip[:, :, :, 2 : 2 + RES] = psi
    v = psip.reshape(NCORES, IPC, 4, 2, RES, XP)
    v = v.transpose(0, 2, 3, 4, 1, 5)  # [NCORES, 4, 2, RES, IPC, XP]
    return np.ascontiguousarray(v).reshape(NCORES, 4, 128, IPC * XP)


def _pack_psi_b(psi):
    """[BS,H,RES,RES] -> [NCORES, 2, 128, IPC*2*RES] bf16: layout B.
    partition p=(c, y16), free=(sg, st, ybl, i, x); y=(2*yp+ybl)*16+y16."""
    v = psi.astype(ml_dtypes.bfloat16).reshape(
        NCORES, NSG, NST, S8, H, 2, 2, 16, RES)
    # axes: 0 core, 1 sg, 2 st, 3 i, 4 c, 5 yp, 6 ybl, 7 y16, 8 x
    v = v.transpose(0, 5, 4, 7, 1, 2, 6, 3, 8)
    return np.ascontiguousarray(v).reshape(NCORES, 2, 128, IPC * 2 * RES)


def _unpack_out(parts):
    """list of [2,128,IPC*2*RES] bf16 per core -> [BS,H,RES,RES] fp32."""
    v = np.stack(parts).reshape(NCORES, 2, H, 16, NSG, NST, 2, S8, RES)
    # axes: 0 core, 1 yp, 2 c, 3 y16, 4 sg, 5 st, 6 ybl, 7 i, 8 x
    v = v.transpose(0, 4, 5, 7, 2, 1, 6, 3, 8)
    return np.ascontiguousarray(v).reshape(BS, H, RES, RES).astype(np.float32)


def kernel(psi, filter1, bias1, w2, b2, w3, b3):
    from concourse.bass_utils import run_bass_kernel_spmd

    psi = np.asarray(psi, dtype=np.float32)
    wmb, bvv = _host_pack(
        np.asarray(filter1, np.float32),
        np.asarray(bias1, np.float32),
        np.asarray(w2, np.float32),
        np.asarray(b2, np.float32),
        np.asarray(w3, np.float32),
        np.asarray(b3, np.float32),
    )

    pat = _pack_psi_a(psi)
    pbt = _pack_psi_b(psi)

    if "nc" not in _CACHE:
        _CACHE["nc"] = _build_program()
    nc = _CACHE["nc"]

    in_maps = [
        {"pa": pat[c], "pb": pbt[c], "wm": wmb, "bv": bvv} for c in range(NCORES)
    ]
    res = run_bass_kernel_spmd(nc, in_maps, list(range(NCORES)))
    return _unpack_out([r["out"] for r in res.results])


# revision 3
# speedup vs baseline: 1.4059x; 1.4059x over previous
"""Trainium2 Bass kernel for the nn_CA depthwise-conv CA step (v3, bf16).

Pipeline per image: depthwise 5x5 conv (D4-symmetrized, zero-mean kernel,
SAME padding) + leaky_relu (bias1==0 in this problem); 1x1 conv (8x8
channel GEMM) + bias + leaky_relu; 1x1 conv + bias + psi residual + tanh.

Strategy: pure data parallel over 8 NeuronCores (256 images each), bf16
data (tolerance 2e-2 rel).

Two on-chip layouts:
  A: partition p=(b, y) for channel pair q (2q+b), free=(image, x). Conv
     stage runs here: 3 matmuls per pair (banded-Toeplitz lhsT for the
     y-conv; D4 x-symmetry folds the 5 x-taps into center + s13 + s2,
     where s13 = psi(x-1)+psi(x+1), s2 = psi(x-2)+psi(x+2) are computed
     on the DVE from the x-padded psi tile — no extra DMA).
  B: partition p=(c, y16), free=(ybl, image, x) per yb-pair tile. Stage2's
     lhsT permutes A->B for free; stage3 then contracts all 8 channels in
     ONE matmul per 512-col tile (4 instead of 16). Residual psi is DMA'd
     a second time in layout B (bf16, cheap) and added on the DVE.

ACT work is merged into 1024-col activations (6 per S8-step).
"""

import numpy as np
import ml_dtypes

BS, H, RES = 2048, 8, 64
NCORES = 8
IPC = BS // NCORES  # images per core
SG = 16             # images per super-group (DMA granularity)
NSG = IPC // SG
S8 = 8              # images per PSUM step (512 free columns)
NST = SG // S8
XP = RES + 4        # x-padded width
NW = 29             # lhsT matrices

_CACHE = {}


def _totalistic(x):
    # D4-symmetrize 5x5 kernels over spatial dims, then remove spatial mean
    z = 0.125 * (x + x[:, :, ::-1, :] + x[:, :, :, ::-1] + x[:, :, ::-1, ::-1])
    xt = np.swapaxes(x, 2, 3)
    z = z + 0.125 * (xt + xt[:, :, ::-1, :] + xt[:, :, :, ::-1] + xt[:, :, ::-1, ::-1])
    return z - z.mean(axis=(2, 3), keepdims=True)


def _build_program(reps=1):
    import concourse.bacc as bacc
    import concourse.tile as tile
    from concourse import mybir

    dt = mybir.dt
    nc = bacc.Bacc("TRN2", target_bir_lowering=False, debug=False, num_devices=NCORES)
    pa = nc.dram_tensor("pa", [4, 128, IPC * XP], dt.bfloat16, kind="ExternalInput").ap()
    pb = nc.dram_tensor("pb", [2, 128, IPC * 2 * RES], dt.bfloat16, kind="ExternalInput").ap()
    wm = nc.dram_tensor("wm", [NW, 128, 128], dt.bfloat16, kind="ExternalInput").ap()
    bv = nc.dram_tensor("bv", [128, 2], dt.float32, kind="ExternalInput").ap()
    out = nc.dram_tensor("out", [2, 128, IPC * 2 * RES], dt.bfloat16, kind="ExternalOutput").ap()

    LR = mybir.ActivationFunctionType.Lrelu
    TH = mybir.ActivationFunctionType.Tanh

    with tile.TileContext(nc) as tc:
        from contextlib import ExitStack

        with ExitStack() as ctx:
            const = ctx.enter_context(tc.tile_pool(name="const", bufs=1))
            psip = ctx.enter_context(tc.tile_pool(name="psip", bufs=2))
            spool = ctx.enter_context(tc.tile_pool(name="spool", bufs=2))
            opool = ctx.enter_context(tc.tile_pool(name="opool", bufs=2))
            zpool = ctx.enter_context(tc.tile_pool(name="zpool", bufs=2))
            psum = ctx.enter_context(tc.tile_pool(name="psum", bufs=4, space="PSUM"))

            wt = const.tile([128, NW * 128], dt.bfloat16)
            nc.sync.dma_start(
                wt[:].rearrange("p (m k) -> p m k", m=NW),
                wm.rearrange("m p k -> p m k"),
            )
            bt = const.tile([128, 2], dt.float32)
            nc.sync.dma_start(bt[:], bv[:])

            def W(i):
                return wt[:, i * 128 : (i + 1) * 128]

            for rep in range(reps):
              for sg in range(NSG):
                ptiles = []
                for q in range(4):
                    t = psip.tile([128, SG * XP], dt.bfloat16, tag=f"psi{q}",
                                  name=f"pa_t{q}_{sg}_{rep}")
                    nc.sync.dma_start(
                        t[:], pa[q, :, sg * SG * XP : (sg + 1) * SG * XP])
                    ptiles.append(t)
                pbtiles = []
                for yp in range(2):
                    t = psip.tile([128, SG * 2 * RES], dt.bfloat16, tag=f"pb{yp}",
                                  name=f"pb_t{yp}_{sg}_{rep}")
                    nc.sync.dma_start(
                        t[:], pb[yp, :, sg * SG * 2 * RES : (sg + 1) * SG * 2 * RES])
                    pbtiles.append(t)
                # x-symmetric partial sums on DVE (replaces the s2 DMA input)
                stiles = []
                for q in range(4):
                    pa3 = ptiles[q][:].rearrange("p (i x) -> p i x", i=SG)
                    s13 = spool.tile([128, SG * RES], dt.bfloat16, tag=f"s13_{q}",
                                     name=f"s13_{q}_{sg}_{rep}")
                    nc.vector.tensor_add(
                        s13[:].rearrange("p (i x) -> p i x", i=SG),
                        pa3[:, :, 1 : 1 + RES], pa3[:, :, 3 : 3 + RES])
                    s2 = spool.tile([128, SG * RES], dt.bfloat16, tag=f"s2_{q}",
                                    name=f"s2_{q}_{sg}_{rep}")
                    nc.gpsimd.tensor_add(
                        s2[:].rearrange("p (i x) -> p i x", i=SG),
                        pa3[:, :, 0 : 0 + RES], pa3[:, :, 4 : 4 + RES])
                    stiles.append((s13, s2))
                otiles = [
                    opool.tile([128, SG * 2 * RES], dt.bfloat16, tag=f"o{yp}",
                               name=f"o_t{yp}_{sg}_{rep}")
                    for yp in range(2)
                ]
                for st in range(NST):
                    # stage 1: depthwise conv + lrelu (layout A)
                    z1half = []
                    for jj in range(2):
                        ps1 = psum.tile([128, 2 * S8 * RES], dt.float32, tag="ps",
                                        name=f"p1_{sg}_{st}_{jj}_{rep}")
                        for h in range(2):
                            q = 2 * jj + h
                            seg = ps1[:, h * S8 * RES : (h + 1) * S8 * RES].rearrange(
                                "p (i x) -> p i x", i=S8)
                            pt3 = ptiles[q][:].rearrange("p (i x) -> p i x", i=SG)[
                                :, st * S8 : (st + 1) * S8, :]
                            s13v = stiles[q][0][:].rearrange("p (i x) -> p i x", i=SG)[
                                :, st * S8 : (st + 1) * S8, :]
                            s2v = stiles[q][1][:].rearrange("p (i x) -> p i x", i=SG)[
                                :, st * S8 : (st + 1) * S8, :]
                            nc.tensor.matmul(seg, W(q * 3 + 2),
                                             pt3[:, :, 2 : 2 + RES],
                                             start=True, stop=False)
                            nc.tensor.matmul(seg, W(q * 3 + 1), s13v,
                                             start=False, stop=False)
                            nc.tensor.matmul(seg, W(q * 3 + 0), s2v,
                                             start=False, stop=True)
                        z = zpool.tile([128, 2 * S8 * RES], dt.bfloat16,
                                       tag=f"z1_{jj}", name=f"z1_{sg}_{st}_{jj}_{rep}")
                        nc.scalar.activation(z[:], ps1[:], LR, alpha=0.01)
                        z1half.append(z)
                    z1 = [
                        z1half[q // 2][:, (q % 2) * S8 * RES : (q % 2 + 1) * S8 * RES]
                        for q in range(4)
                    ]
                    # stage 2: 8x8 GEMM, layout A -> B via lhsT, + b2 + lrelu
                    z2t = []
                    for yp in range(2):
                        ps2 = psum.tile([128, 2 * S8 * RES], dt.float32, tag="ps",
                                        name=f"p2_{sg}_{st}_{yp}_{rep}")
                        for ybl in range(2):
                            yb = 2 * yp + ybl
                            seg = ps2[:, ybl * S8 * RES : (ybl + 1) * S8 * RES]
                            for q in range(4):
                                nc.tensor.matmul(seg, W(12 + yb * 4 + q), z1[q],
                                                 start=(q == 0), stop=(q == 3))
                        z = zpool.tile([128, 2 * S8 * RES], dt.bfloat16,
                                       tag=f"z2_{yp}", name=f"z2_{sg}_{st}_{yp}_{rep}")
                        nc.scalar.activation(z[:], ps2[:], LR,
                                             bias=bt[:, 0:1], alpha=0.01)
                        z2t.append(z)
                    # stage 3: 8x8 GEMM in layout B (1 matmul per 512-col tile)
                    # + psi residual (DVE) + b3 + tanh
                    for yp in range(2):
                        ps3 = psum.tile([128, 2 * S8 * RES], dt.float32, tag="ps",
                                        name=f"p3_{sg}_{st}_{yp}_{rep}")
                        for ybl in range(2):
                            seg = ps3[:, ybl * S8 * RES : (ybl + 1) * S8 * RES]
                            nc.tensor.matmul(
                                seg, W(28),
                                z2t[yp][:, ybl * S8 * RES : (ybl + 1) * S8 * RES],
                                start=True, stop=True)
                        pbv = pbtiles[yp][:, st * 2 * S8 * RES : (st + 1) * 2 * S8 * RES]
                        nc.vector.tensor_add(ps3[:], ps3[:], pbv)
                        nc.scalar.activation(
                            otiles[yp][:, st * 2 * S8 * RES : (st + 1) * 2 * S8 * RES],
                            ps3[:], TH, bias=bt[:, 1:2])
                for yp in range(2):
                    nc.sync.dma_start(
                        out[yp, :, sg * SG * 2 * RES : (sg + 1) * SG * 2 * RES],
                        otiles[yp][:])

    nc.compile()
    return nc


def _host_pack(filter1, bias1, w2, b2, w3, b3):
    w = _totalistic(filter1.astype(np.float32))[:, 0]  # [8,5,5]
    wm = np.zeros((NW, 128, 128), np.float32)
    eye = {d: np.eye(RES, k=-d, dtype=np.float32) for d in range(-2, 3)}
    for q in range(4):
        for dxi in range(3):
            m = wm[q * 3 + dxi]
            for b in range(2):
                blk = np.zeros((RES, RES), np.float32)
                for d in range(-2, 3):
                    blk += w[2 * q + b, d + 2, dxi] * eye[d]
                m[b * 64 : b * 64 + 64, b * 64 : b * 64 + 64] = blk
    # stage2: lhsT[(b, y64), (co, y16)] = w2[co, 2q+b] iff y64 == 16*yb + y16
    for yb in range(4):
        for q in range(4):
            m = wm[12 + yb * 4 + q]
            for b in range(2):
                for co in range(H):
                    for y16 in range(16):
                        m[b * 64 + 16 * yb + y16, co * 16 + y16] = w2[co, 2 * q + b]
    # stage3: lhsT[(ci, y16), (co, y16)] = w3[co, ci]
    m = wm[28]
    for ci in range(H):
        for co in range(H):
            for y16 in range(16):
                m[ci * 16 + y16, co * 16 + y16] = w3[co, ci]

    bvv = np.zeros((128, 2), np.float32)
    for c in range(H):
        bvv[c * 16 : c * 16 + 16, 0] = b2[c]
        bvv[c * 16 : c * 16 + 16, 1] = b3[c]
    return wm.astype(ml_dtypes.bfloat16), bvv


def _pack_psi_a(psi):
    """[BS,H,RES,RES] -> [NCORES, 4, 128, IPC*XP] bf16: partition p=(b,y) of
    channel pair q, free = (image, padded x)."""
    psip = np.zeros((BS, H, RES, XP), ml_dtypes.bfloat16)
    psip[:, :, :, 2 : 2 + RES] = psi
    v = psip.reshape(NCORES, IPC, 4, 2, RES, XP)
    v = v.transpose(0, 2, 3, 4, 1, 5)  # [NCORES, 4, 2, RES, IPC, XP]
    return np.ascontiguousarray(v).reshape(NCORES, 4, 128, IPC * XP)


def _pack_psi_b(psi):
    """[BS,H,RES,RES] -> [NCORES, 2, 128, IPC*2*RES] bf16: layout B.
    partition p=(c, y16), free=(sg, st, ybl, i, x); y=(2*yp+ybl)*16+y16."""
    v = psi.astype(ml_dtypes.bfloat16).reshape(
        NCORES, NSG, NST, S8, H, 2, 2, 16, RES)
    # axes: 0 core, 1 sg, 2 st, 3 i, 4 c, 5 yp, 6 ybl, 7 y16, 8 x
    v = v.transpose(0, 5, 4, 7, 1, 2, 6, 3, 8)
    return np.ascontiguousarray(v).reshape(NCORES, 2, 128, IPC * 2 * RES)


def _unpack_out(parts):
    """list of [2,128,IPC*2*RES] bf16 per core -> [BS,H,RES,RES] fp32."""
    v = np.stack(parts).reshape(NCORES, 2, H, 16, NSG, NST, 2, S8, RES)
    # axes: 0 core, 1 yp, 2 c, 3 y16, 4 sg, 5 st, 6 ybl, 7 i, 8 x
    v = v.transpose(0, 4, 5, 7, 2, 1, 6, 3, 8)
    return np.ascontiguousarray(v).reshape(BS, H, RES, RES).astype(np.float32)


def kernel(psi, filter1, bias1, w2, b2, w3, b3):
    from concourse.bass_utils import run_bass_kernel_spmd

    psi = np.asarray(psi, dtype=np.float32)
    wmb, bvv = _host_pack(
        np.asarray(filter1, np.float32),
        np.asarray(bias1, np.float32),
        np.asarray(w2, np.float32),
        np.asarray(b2, np.float32),
        np.asarray(w3, np.float32),
        np.asarray(b3, np.float32),
    )

    pat = _pack_psi_a(psi)
    pbt = _pack_psi_b(psi)

    if "nc" not in _CACHE:
        _CACHE["nc"] = _build_program()
    nc = _CACHE["nc"]

    in_maps = [
        {"pa": pat[c], "pb": pbt[c], "wm": wmb, "bv": bvv} for c in range(NCORES)
    ]
    res = run_bass_kernel_spmd(nc, in_maps, list(range(NCORES)))
    return _unpack_out([r["out"] for r in res.results])


# revision 15
# speedup vs baseline: 1.8231x; 1.2967x over previous
"""Trainium2 Bass kernel for the nn_CA depthwise-conv CA step (v3, bf16).

Pipeline per image: depthwise 5x5 conv (D4-symmetrized, zero-mean kernel,
SAME padding) + leaky_relu (bias1==0 in this problem); 1x1 conv (8x8
channel GEMM) + bias + leaky_relu; 1x1 conv + bias + psi residual + tanh.

Strategy: pure data parallel over 8 NeuronCores (256 images each), bf16
data (tolerance 2e-2 rel).

Two on-chip layouts:
  A: partition p=(b, y) for channel pair q (2q+b), free=(image, x). Conv
     stage runs here: 3 matmuls per pair (banded-Toeplitz lhsT for the
     y-conv; D4 x-symmetry folds the 5 x-taps into center + s13 + s2,
     where s13 = psi(x-1)+psi(x+1), s2 = psi(x-2)+psi(x+2) are computed
     on the DVE from the x-padded psi tile — no extra DMA).
  B: partition p=(c, y16), free=(ybl, image, x) per yb-pair tile. Stage2's
     lhsT permutes A->B for free; stage3 then contracts all 8 channels in
     ONE matmul per 512-col tile (4 instead of 16). Residual psi is DMA'd
     a second time in layout B (bf16, cheap) and added on the DVE.

ACT work is merged into 1024-col activations (6 per S8-step).
"""

import numpy as np
import ml_dtypes

BS, H, RES = 2048, 8, 64
NCORES = 8
IPC = BS // NCORES  # images per core
SG = 16             # images per super-group (DMA granularity)
NSG = IPC // SG
S8 = 8              # images per PSUM step (512 free columns)
NST = SG // S8
XP = RES + 4        # x-padded width
NW = 29             # lhsT matrices

_CACHE = {}


def _totalistic(x):
    # D4-symmetrize 5x5 kernels over spatial dims, then remove spatial mean
    z = 0.125 * (x + x[:, :, ::-1, :] + x[:, :, :, ::-1] + x[:, :, ::-1, ::-1])
    xt = np.swapaxes(x, 2, 3)
    z = z + 0.125 * (xt + xt[:, :, ::-1, :] + xt[:, :, :, ::-1] + xt[:, :, ::-1, ::-1])
    return z - z.mean(axis=(2, 3), keepdims=True)


def _build_program(reps=1):
    import concourse.bacc as bacc
    import concourse.tile as tile
    from concourse import mybir

    dt = mybir.dt
    from concourse.alu_op_type import AluOpType
    nc = bacc.Bacc("TRN2", target_bir_lowering=False, debug=False, num_devices=NCORES)
    pa = nc.dram_tensor("pa", [128, NSG * 4 * SG * XP], dt.bfloat16, kind="ExternalInput").ap()
    pb = nc.dram_tensor("pb", [128, IPC * 4 * RES], dt.bfloat16, kind="ExternalInput").ap()
    wm = nc.dram_tensor("wm", [NW, 128, 128], dt.bfloat16, kind="ExternalInput").ap()
    bv = nc.dram_tensor("bv", [128, 2], dt.float32, kind="ExternalInput").ap()
    out = nc.dram_tensor("out", [128, IPC * 4 * RES], dt.bfloat16, kind="ExternalOutput").ap()

    LR = mybir.ActivationFunctionType.Lrelu
    TH = mybir.ActivationFunctionType.Tanh

    with tile.TileContext(nc) as tc:
        from contextlib import ExitStack

        with ExitStack() as ctx:
            const = ctx.enter_context(tc.tile_pool(name="const", bufs=1))
            psip = ctx.enter_context(tc.tile_pool(name="psip", bufs=2))
            spool = ctx.enter_context(tc.tile_pool(name="spool", bufs=2))
            opool = ctx.enter_context(tc.tile_pool(name="opool", bufs=2))
            zpool = ctx.enter_context(tc.tile_pool(name="zpool", bufs=2))
            psum = ctx.enter_context(tc.tile_pool(name="psum", bufs=4, space="PSUM"))

            wt = const.tile([128, NW * 128], dt.bfloat16)
            nc.sync.dma_start(
                wt[:].rearrange("p (m k) -> p m k", m=NW),
                wm.rearrange("m p k -> p m k"),
            )
            bt = const.tile([128, 2], dt.float32)
            nc.sync.dma_start(bt[:], bv[:])

            def W(i):
                return wt[:, i * 128 : (i + 1) * 128]

            for rep in range(reps):
              for sg in range(NSG):
                pat = psip.tile([128, 4 * SG * XP], dt.bfloat16, tag="pa",
                                name=f"pa_t_{sg}_{rep}")
                nc.sync.dma_start(
                    pat[:], pa[:, sg * 4 * SG * XP : (sg + 1) * 4 * SG * XP])
                ptiles = [pat[:, q * SG * XP : (q + 1) * SG * XP] for q in range(4)]
                pbt = psip.tile([128, SG * 4 * RES], dt.bfloat16, tag="pb",
                                name=f"pb_t_{sg}_{rep}")
                nc.sync.dma_start(
                    pbt[:], pb[:, sg * SG * 4 * RES : (sg + 1) * SG * 4 * RES])
                # x-symmetric partial sums on DVE/GpSimd (replace the s2 DMA)
                stiles = []
                for q in range(4):
                    pa3 = ptiles[q].rearrange("p (i x) -> p i x", i=SG)
                    s13 = spool.tile([128, SG * RES], dt.bfloat16, tag=f"s13_{q}",
                                     name=f"s13_{q}_{sg}_{rep}")
                    nc.vector.tensor_add(
                        s13[:].rearrange("p (i x) -> p i x", i=SG),
                        pa3[:, :, 1 : 1 + RES], pa3[:, :, 3 : 3 + RES])
                    s2 = spool.tile([128, SG * RES], dt.bfloat16, tag=f"s2_{q}",
                                    name=f"s2_{q}_{sg}_{rep}")
                    nc.gpsimd.tensor_add(
                        s2[:].rearrange("p (i x) -> p i x", i=SG),
                        pa3[:, :, 0 : 0 + RES], pa3[:, :, 4 : 4 + RES])
                    stiles.append((s13, s2))
                otile = opool.tile([128, SG * 4 * RES], dt.bfloat16, tag="o",
                                   name=f"o_t_{sg}_{rep}")
                for st in range(NST):
                    # stage 1: depthwise conv + lrelu (layout A)
                    z1half = []
                    for jj in range(2):
                        ps1 = psum.tile([128, 2 * S8 * RES], dt.float32, tag="ps",
                                        name=f"p1_{sg}_{st}_{jj}_{rep}")
                        for h in range(2):
                            q = 2 * jj + h
                            seg = ps1[:, h * S8 * RES : (h + 1) * S8 * RES].rearrange(
                                "p (i x) -> p i x", i=S8)
                            pt3 = ptiles[q].rearrange("p (i x) -> p i x", i=SG)[
                                :, st * S8 : (st + 1) * S8, :]
                            s13v = stiles[q][0][:].rearrange("p (i x) -> p i x", i=SG)[
                                :, st * S8 : (st + 1) * S8, :]
                            s2v = stiles[q][1][:].rearrange("p (i x) -> p i x", i=SG)[
                                :, st * S8 : (st + 1) * S8, :]
                            nc.tensor.matmul(seg, W(q * 3 + 2),
                                             pt3[:, :, 2 : 2 + RES],
                                             start=True, stop=False)
                            nc.tensor.matmul(seg, W(q * 3 + 1), s13v,
                                             start=False, stop=False)
                            nc.tensor.matmul(seg, W(q * 3 + 0), s2v,
                                             start=False, stop=True)
                        z = zpool.tile([128, 2 * S8 * RES], dt.bfloat16,
                                       tag=f"z1_{jj}", name=f"z1_{sg}_{st}_{jj}_{rep}")
                        nc.scalar.activation(z[:], ps1[:], LR, alpha=0.01)
                        z1half.append(z)
                    z1 = [
                        z1half[q // 2][:, (q % 2) * S8 * RES : (q % 2 + 1) * S8 * RES]
                        for q in range(4)
                    ]
                    # stage 2: 8x8 GEMM, layout A -> B via lhsT, + b2 + lrelu
                    z2t = []
                    for yp in range(2):
                        ps2 = psum.tile([128, 2 * S8 * RES], dt.float32, tag="ps",
                                        name=f"p2_{sg}_{st}_{yp}_{rep}")
                        for ybl in range(2):
                            yb = 2 * yp + ybl
                            seg = ps2[:, ybl * S8 * RES : (ybl + 1) * S8 * RES]
                            for q in range(4):
                                nc.tensor.matmul(seg, W(12 + yb * 4 + q), z1[q],
                                                 start=(q == 0), stop=(q == 3))
                        z = zpool.tile([128, 2 * S8 * RES], dt.bfloat16,
                                       tag=f"z2_{yp}", name=f"z2_{sg}_{st}_{yp}_{rep}")
                        nc.scalar.activation(z[:], ps2[:], LR,
                                             bias=bt[:, 0:1], alpha=0.01)
                        z2t.append(z)
                    # stage 3: 8x8 GEMM in layout B (1 matmul per 512-col tile)
                    # + psi residual (DVE) + b3 + tanh
                    for yp in range(2):
                        ps3 = psum.tile([128, 2 * S8 * RES], dt.float32, tag="ps",
                                        name=f"p3_{sg}_{st}_{yp}_{rep}")
                        for ybl in range(2):
                            seg = ps3[:, ybl * S8 * RES : (ybl + 1) * S8 * RES]
                            nc.tensor.matmul(
                                seg, W(28),
                                z2t[yp][:, ybl * S8 * RES : (ybl + 1) * S8 * RES],
                                start=True, stop=True)
                        off = st * 4 * S8 * RES + yp * 2 * S8 * RES
                        pbv = pbt[:, off : off + 2 * S8 * RES]
                        nc.vector.tensor_add(ps3[:], ps3[:], pbv)
                        nc.scalar.activation(
                            otile[:, off : off + 2 * S8 * RES],
                            ps3[:], TH, bias=bt[:, 1:2])
                nc.sync.dma_start(
                    out[:, sg * SG * 4 * RES : (sg + 1) * SG * 4 * RES],
                    otile[:])

    nc.compile()
    return nc


def _host_pack(filter1, bias1, w2, b2, w3, b3):
    w = _totalistic(filter1.astype(np.float32))[:, 0]  # [8,5,5]
    wm = np.zeros((NW, 128, 128), np.float32)
    eye = {d: np.eye(RES, k=-d, dtype=np.float32) for d in range(-2, 3)}
    for q in range(4):
        for dxi in range(3):
            m = wm[q * 3 + dxi]
            for b in range(2):
                blk = np.zeros((RES, RES), np.float32)
                for d in range(-2, 3):
                    blk += w[2 * q + b, d + 2, dxi] * eye[d]
                m[b * 64 : b * 64 + 64, b * 64 : b * 64 + 64] = blk
    # stage2: lhsT[(b, y64), (co, y16)] = w2[co, 2q+b] iff y64 == 16*yb + y16
    for yb in range(4):
        for q in range(4):
            m = wm[12 + yb * 4 + q]
            for b in range(2):
                for co in range(H):
                    for y16 in range(16):
                        m[b * 64 + 16 * yb + y16, co * 16 + y16] = w2[co, 2 * q + b]
    # stage3: lhsT[(ci, y16), (co, y16)] = w3[co, ci]
    m = wm[28]
    for ci in range(H):
        for co in range(H):
            for y16 in range(16):
                m[ci * 16 + y16, co * 16 + y16] = w3[co, ci]

    bvv = np.zeros((128, 2), np.float32)
    for c in range(H):
        bvv[c * 16 : c * 16 + 16, 0] = b2[c]
        bvv[c * 16 : c * 16 + 16, 1] = b3[c]
    return wm.astype(ml_dtypes.bfloat16), bvv


def _pack_psi_a(psi):
    """[BS,H,RES,RES] -> [NCORES, 128, NSG*4*SG*XP] bf16: partition p=(b,y)
    of channel pair q, free = (sg, q, image-in-sg, padded x)."""
    psip = np.zeros((BS, H, RES, XP), ml_dtypes.bfloat16)
    psip[:, :, :, 2 : 2 + RES] = psi
    v = psip.reshape(NCORES, NSG, SG, 4, 2, RES, XP)
    # -> [core, b(2), y(RES), sg, q, i, xp]
    v = v.transpose(0, 4, 5, 1, 3, 2, 6)
    return np.ascontiguousarray(v).reshape(NCORES, 128, NSG * 4 * SG * XP)


def _pack_psi_b(psi):
    """[BS,H,RES,RES] -> [NCORES, 128, IPC*4*RES] bf16: layout B.
    partition p=(c, y16), free=(sg, st, yb, i, x); y = yb*16 + y16."""
    v = psi.astype(ml_dtypes.bfloat16).reshape(
        NCORES, NSG, NST, S8, H, 4, 16, RES)
    # axes: 0 core, 1 sg, 2 st, 3 i, 4 c, 5 yb, 6 y16, 7 x
    v = v.transpose(0, 4, 6, 1, 2, 5, 3, 7)
    return np.ascontiguousarray(v).reshape(NCORES, 128, IPC * 4 * RES)


def _unpack_out(parts):
    """list of [128, IPC*4*RES] bf16 per core -> [BS,H,RES,RES] fp32."""
    v = np.stack(parts).reshape(NCORES, H, 16, NSG, NST, 4, S8, RES)
    # axes: 0 core, 1 c, 2 y16, 3 sg, 4 st, 5 yb, 6 i, 7 x
    v = v.transpose(0, 3, 4, 6, 1, 5, 2, 7)
    return np.ascontiguousarray(v).reshape(BS, H, RES, RES).astype(np.float32)


def kernel(psi, filter1, bias1, w2, b2, w3, b3):
    from concourse.bass_utils import run_bass_kernel_spmd

    psi = np.asarray(psi, dtype=np.float32)
    wmb, bvv = _host_pack(
        np.asarray(filter1, np.float32),
        np.asarray(bias1, np.float32),
        np.asarray(w2, np.float32),
        np.asarray(b2, np.float32),
        np.asarray(w3, np.float32),
        np.asarray(b3, np.float32),
    )

    pat = _pack_psi_a(psi)
    pbt = _pack_psi_b(psi)

    if "nc" not in _CACHE:
        _CACHE["nc"] = _build_program()
    nc = _CACHE["nc"]

    in_maps = [
        {"pa": pat[c], "pb": pbt[c], "wm": wmb, "bv": bvv} for c in range(NCORES)
    ]
    res = run_bass_kernel_spmd(nc, in_maps, list(range(NCORES)))
    return _unpack_out([r["out"] for r in res.results])


# revision 19
# speedup vs baseline: 2.0640x; 1.1322x over previous
"""Trainium2 Bass kernel for the nn_CA depthwise-conv CA step (v3, bf16).

Pipeline per image: depthwise 5x5 conv (D4-symmetrized, zero-mean kernel,
SAME padding) + leaky_relu (bias1==0 in this problem); 1x1 conv (8x8
channel GEMM) + bias + leaky_relu; 1x1 conv + bias + psi residual + tanh.

Strategy: pure data parallel over 8 NeuronCores (256 images each), bf16
data (tolerance 2e-2 rel).

Two on-chip layouts:
  A: partition p=(b, y) for channel pair q (2q+b), free=(image, x). Conv
     stage runs here: 3 matmuls per pair (banded-Toeplitz lhsT for the
     y-conv; D4 x-symmetry folds the 5 x-taps into center + s13 + s2,
     where s13 = psi(x-1)+psi(x+1), s2 = psi(x-2)+psi(x+2) are computed
     on the DVE from the x-padded psi tile — no extra DMA).
  B: partition p=(c, y16), free=(ybl, image, x) per yb-pair tile. Stage2's
     lhsT permutes A->B for free; stage3 then contracts all 8 channels in
     ONE matmul per 512-col tile (4 instead of 16). Residual psi is DMA'd
     a second time in layout B (bf16, cheap) and added on the DVE.

ACT work is merged into 1024-col activations (6 per S8-step).
"""

import numpy as np
import ml_dtypes

BS, H, RES = 2048, 8, 64
NCORES = 8
IPC = BS // NCORES  # images per core
SG = 16             # images per super-group (DMA granularity)
NSG = IPC // SG
S8 = 8              # images per PSUM step (512 free columns)
NST = SG // S8
XP = RES + 4        # x-padded width
NW = 30             # lhsT matrices (29 = I128 for the PE-side psi residual)

_CACHE = {}


def _totalistic(x):
    # D4-symmetrize 5x5 kernels over spatial dims, then remove spatial mean
    z = 0.125 * (x + x[:, :, ::-1, :] + x[:, :, :, ::-1] + x[:, :, ::-1, ::-1])
    xt = np.swapaxes(x, 2, 3)
    z = z + 0.125 * (xt + xt[:, :, ::-1, :] + xt[:, :, :, ::-1] + xt[:, :, ::-1, ::-1])
    return z - z.mean(axis=(2, 3), keepdims=True)


def _build_program(reps=1):
    import concourse.bacc as bacc
    import concourse.tile as tile
    from concourse import mybir

    dt = mybir.dt
    from concourse.alu_op_type import AluOpType
    nc = bacc.Bacc("TRN2", target_bir_lowering=False, debug=False, num_devices=NCORES)
    pa = nc.dram_tensor("pa", [128, NSG * 4 * SG * XP], dt.bfloat16, kind="ExternalInput").ap()
    pb = nc.dram_tensor("pb", [128, IPC * 4 * RES], dt.bfloat16, kind="ExternalInput").ap()
    wm = nc.dram_tensor("wm", [NW, 128, 128], dt.bfloat16, kind="ExternalInput").ap()
    bv = nc.dram_tensor("bv", [128, 2], dt.float32, kind="ExternalInput").ap()
    out = nc.dram_tensor("out", [128, IPC * 4 * RES], dt.bfloat16, kind="ExternalOutput").ap()

    LR = mybir.ActivationFunctionType.Lrelu
    TH = mybir.ActivationFunctionType.Tanh

    with tile.TileContext(nc) as tc:
        from contextlib import ExitStack

        with ExitStack() as ctx:
            const = ctx.enter_context(tc.tile_pool(name="const", bufs=1))
            psip = ctx.enter_context(tc.tile_pool(name="psip", bufs=2))
            spool = ctx.enter_context(tc.tile_pool(name="spool", bufs=2))
            opool = ctx.enter_context(tc.tile_pool(name="opool", bufs=2))
            zpool = ctx.enter_context(tc.tile_pool(name="zpool", bufs=2))
            psum = ctx.enter_context(tc.tile_pool(name="psum", bufs=4, space="PSUM"))

            wt = const.tile([128, NW * 128], dt.bfloat16)
            nc.sync.dma_start(
                wt[:].rearrange("p (m k) -> p m k", m=NW),
                wm.rearrange("m p k -> p m k"),
            )
            bt = const.tile([128, 2], dt.float32)
            nc.sync.dma_start(bt[:], bv[:])

            def W(i):
                return wt[:, i * 128 : (i + 1) * 128]

            for rep in range(reps):
              for sg in range(NSG):
                pat = psip.tile([128, 4 * SG * XP], dt.bfloat16, tag="pa",
                                name=f"pa_t_{sg}_{rep}")
                nc.sync.dma_start(
                    pat[:], pa[:, sg * 4 * SG * XP : (sg + 1) * 4 * SG * XP])
                ptiles = [pat[:, q * SG * XP : (q + 1) * SG * XP] for q in range(4)]
                pbt = psip.tile([128, SG * 4 * RES], dt.bfloat16, tag="pb",
                                name=f"pb_t_{sg}_{rep}")
                nc.sync.dma_start(
                    pbt[:], pb[:, sg * SG * 4 * RES : (sg + 1) * SG * 4 * RES])
                # x-symmetric partial sums on DVE/GpSimd (replace the s2 DMA)
                stiles = []
                for q in range(4):
                    pa3 = ptiles[q].rearrange("p (i x) -> p i x", i=SG)
                    s13 = spool.tile([128, SG * RES], dt.bfloat16, tag=f"s13_{q}",
                                     name=f"s13_{q}_{sg}_{rep}")
                    nc.vector.tensor_add(
                        s13[:].rearrange("p (i x) -> p i x", i=SG),
                        pa3[:, :, 1 : 1 + RES], pa3[:, :, 3 : 3 + RES])
                    s2 = spool.tile([128, SG * RES], dt.bfloat16, tag=f"s2_{q}",
                                    name=f"s2_{q}_{sg}_{rep}")
                    nc.gpsimd.tensor_add(
                        s2[:].rearrange("p (i x) -> p i x", i=SG),
                        pa3[:, :, 0 : 0 + RES], pa3[:, :, 4 : 4 + RES])
                    stiles.append((s13, s2))
                otile = opool.tile([128, SG * 4 * RES], dt.bfloat16, tag="o",
                                   name=f"o_t_{sg}_{rep}")
                for st in range(NST):
                    # stage 1: depthwise conv + lrelu (layout A)
                    z1half = []
                    for jj in range(2):
                        ps1 = psum.tile([128, 2 * S8 * RES], dt.float32, tag="ps",
                                        name=f"p1_{sg}_{st}_{jj}_{rep}")
                        for h in range(2):
                            q = 2 * jj + h
                            seg = ps1[:, h * S8 * RES : (h + 1) * S8 * RES].rearrange(
                                "p (i x) -> p i x", i=S8)
                            pt3 = ptiles[q].rearrange("p (i x) -> p i x", i=SG)[
                                :, st * S8 : (st + 1) * S8, :]
                            s13v = stiles[q][0][:].rearrange("p (i x) -> p i x", i=SG)[
                                :, st * S8 : (st + 1) * S8, :]
                            s2v = stiles[q][1][:].rearrange("p (i x) -> p i x", i=SG)[
                                :, st * S8 : (st + 1) * S8, :]
                            nc.tensor.matmul(seg, W(q * 3 + 2),
                                             pt3[:, :, 2 : 2 + RES],
                                             start=True, stop=False)
                            nc.tensor.matmul(seg, W(q * 3 + 1), s13v,
                                             start=False, stop=False)
                            nc.tensor.matmul(seg, W(q * 3 + 0), s2v,
                                             start=False, stop=True)
                        z = zpool.tile([128, 2 * S8 * RES], dt.bfloat16,
                                       tag=f"z1_{jj}", name=f"z1_{sg}_{st}_{jj}_{rep}")
                        if jj == 0:
                            # lrelu = max(x, 0.01x) on DVE (ACT is the wall)
                            tmp = zpool.tile([128, 2 * S8 * RES], dt.bfloat16,
                                             tag="t1tmp",
                                             name=f"t1tmp_{sg}_{st}_{rep}")
                            nc.vector.tensor_scalar_mul(tmp[:], ps1[:], 0.01)
                            nc.vector.tensor_max(z[:], ps1[:], tmp[:])
                        else:
                            nc.scalar.activation(z[:], ps1[:], LR, alpha=0.01)
                        z1half.append(z)
                    z1 = [
                        z1half[q // 2][:, (q % 2) * S8 * RES : (q % 2 + 1) * S8 * RES]
                        for q in range(4)
                    ]
                    # stage 2: 8x8 GEMM, layout A -> B via lhsT, + b2 + lrelu
                    z2t = []
                    for yp in range(2):
                        ps2 = psum.tile([128, 2 * S8 * RES], dt.float32, tag="ps",
                                        name=f"p2_{sg}_{st}_{yp}_{rep}")
                        for ybl in range(2):
                            yb = 2 * yp + ybl
                            seg = ps2[:, ybl * S8 * RES : (ybl + 1) * S8 * RES]
                            for q in range(4):
                                nc.tensor.matmul(seg, W(12 + yb * 4 + q), z1[q],
                                                 start=(q == 0), stop=(q == 3))
                        z = zpool.tile([128, 2 * S8 * RES], dt.bfloat16,
                                       tag=f"z2_{yp}", name=f"z2_{sg}_{st}_{yp}_{rep}")
                        nc.scalar.activation(z[:], ps2[:], LR,
                                             bias=bt[:, 0:1], alpha=0.01)
                        z2t.append(z)
                    # stage 3: 8x8 GEMM in layout B (1 matmul per 512-col tile)
                    # + psi residual (DVE) + b3 + tanh
                    for yp in range(2):
                        ps3 = psum.tile([128, 2 * S8 * RES], dt.float32, tag="ps",
                                        name=f"p3_{sg}_{st}_{yp}_{rep}")
                        off = st * 4 * S8 * RES + yp * 2 * S8 * RES
                        for ybl in range(2):
                            seg = ps3[:, ybl * S8 * RES : (ybl + 1) * S8 * RES]
                            # psi residual rides an identity matmul (PE has slack)
                            nc.tensor.matmul(
                                seg, W(29),
                                pbt[:, off + ybl * S8 * RES : off + (ybl + 1) * S8 * RES],
                                start=True, stop=False)
                            nc.tensor.matmul(
                                seg, W(28),
                                z2t[yp][:, ybl * S8 * RES : (ybl + 1) * S8 * RES],
                                start=False, stop=True)
                        nc.scalar.activation(
                            otile[:, off : off + 2 * S8 * RES],
                            ps3[:], TH, bias=bt[:, 1:2])
                nc.sync.dma_start(
                    out[:, sg * SG * 4 * RES : (sg + 1) * SG * 4 * RES],
                    otile[:])

    nc.compile()
    return nc


def _host_pack(filter1, bias1, w2, b2, w3, b3):
    w = _totalistic(filter1.astype(np.float32))[:, 0]  # [8,5,5]
    wm = np.zeros((NW, 128, 128), np.float32)
    eye = {d: np.eye(RES, k=-d, dtype=np.float32) for d in range(-2, 3)}
    for q in range(4):
        for dxi in range(3):
            m = wm[q * 3 + dxi]
            for b in range(2):
                blk = np.zeros((RES, RES), np.float32)
                for d in range(-2, 3):
                    blk += w[2 * q + b, d + 2, dxi] * eye[d]
                m[b * 64 : b * 64 + 64, b * 64 : b * 64 + 64] = blk
    # stage2: lhsT[(b, y64), (co, y16)] = w2[co, 2q+b] iff y64 == 16*yb + y16
    for yb in range(4):
        for q in range(4):
            m = wm[12 + yb * 4 + q]
            for b in range(2):
                for co in range(H):
                    for y16 in range(16):
                        m[b * 64 + 16 * yb + y16, co * 16 + y16] = w2[co, 2 * q + b]
    # stage3: lhsT[(ci, y16), (co, y16)] = w3[co, ci]
    m = wm[28]
    for ci in range(H):
        for co in range(H):
            for y16 in range(16):
                m[ci * 16 + y16, co * 16 + y16] = w3[co, ci]
    wm[29] = np.eye(128, dtype=np.float32)

    bvv = np.zeros((128, 2), np.float32)
    for c in range(H):
        bvv[c * 16 : c * 16 + 16, 0] = b2[c]
        bvv[c * 16 : c * 16 + 16, 1] = b3[c]
    return wm.astype(ml_dtypes.bfloat16), bvv


def _pack_psi_a(psi):
    """[BS,H,RES,RES] -> [NCORES, 128, NSG*4*SG*XP] bf16: partition p=(b,y)
    of channel pair q, free = (sg, q, image-in-sg, padded x)."""
    psip = np.zeros((BS, H, RES, XP), ml_dtypes.bfloat16)
    psip[:, :, :, 2 : 2 + RES] = psi
    v = psip.reshape(NCORES, NSG, SG, 4, 2, RES, XP)
    # -> [core, b(2), y(RES), sg, q, i, xp]
    v = v.transpose(0, 4, 5, 1, 3, 2, 6)
    return np.ascontiguousarray(v).reshape(NCORES, 128, NSG * 4 * SG * XP)


def _pack_psi_b(psi):
    """[BS,H,RES,RES] -> [NCORES, 128, IPC*4*RES] bf16: layout B.
    partition p=(c, y16), free=(sg, st, yb, i, x); y = yb*16 + y16."""
    v = psi.astype(ml_dtypes.bfloat16).reshape(
        NCORES, NSG, NST, S8, H, 4, 16, RES)
    # axes: 0 core, 1 sg, 2 st, 3 i, 4 c, 5 yb, 6 y16, 7 x
    v = v.transpose(0, 4, 6, 1, 2, 5, 3, 7)
    return np.ascontiguousarray(v).reshape(NCORES, 128, IPC * 4 * RES)


def _unpack_out(parts):
    """list of [128, IPC*4*RES] bf16 per core -> [BS,H,RES,RES] fp32."""
    v = np.stack(parts).reshape(NCORES, H, 16, NSG, NST, 4, S8, RES)
    # axes: 0 core, 1 c, 2 y16, 3 sg, 4 st, 5 yb, 6 i, 7 x
    v = v.transpose(0, 3, 4, 6, 1, 5, 2, 7)
    return np.ascontiguousarray(v).reshape(BS, H, RES, RES).astype(np.float32)


def kernel(psi, filter1, bias1, w2, b2, w3, b3):
    from concourse.bass_utils import run_bass_kernel_spmd

    psi = np.asarray(psi, dtype=np.float32)
    wmb, bvv = _host_pack(
        np.asarray(filter1, np.float32),
        np.asarray(bias1, np.float32),
        np.asarray(w2, np.float32),
        np.asarray(b2, np.float32),
        np.asarray(w3, np.float32),
        np.asarray(b3, np.float32),
    )

    pat = _pack_psi_a(psi)
    pbt = _pack_psi_b(psi)

    if "nc" not in _CACHE:
        _CACHE["nc"] = _build_program()
    nc = _CACHE["nc"]

    in_maps = [
        {"pa": pat[c], "pb": pbt[c], "wm": wmb, "bv": bvv} for c in range(NCORES)
    ]
    res = run_bass_kernel_spmd(nc, in_maps, list(range(NCORES)))
    return _unpack_out([r["out"] for r in res.results])
